# revision 33
# baseline (speedup 1.0000x reference)
"""CrossAttention TRN2 Bass kernel — 8-core data-parallel (batch x query-half).

Sharding: core c -> batch b=c//2, query rows [(c%2)*1024, (c%2+1)*1024).
Each core computes its 1024 output rows end-to-end (kv recomputed per
core-pair; no collectives). Host pre-transposes activations so every
matmul operand is contraction-major in DRAM.

kv compaction: masked kv positions are gathered out on the host (pad to
M2C=640 rows of zeros). Zero k rows give exp(0)=1 at pads, but the
ones-column appended to v carries the keep flag, so pads contribute
exactly 0 to both the attention numerator and the softmax denominator.

v2 restructure vs the 290us baseline:
- Scores matmuls for a head PAIR run row-tiled (64x128 mode, tiles
  T0/T8): head 2co lives on SBUF partitions 0-63, head 2co+1 on 64-127
  (the kT/qT layout already interleaves heads that way), so the two
  64-contraction matmuls execute CONCURRENTLY in the two array halves.
- One fused schedule: per co-slot [A2 kproj(co), A1 qproj(co),
  scores+exp(co), attnv(co-1)] so the ACT-engine exp stream (~11us/pair)
  hides under PE work of the same slot; vproj runs up front with
  y-stationary c4-paired matmuls (80 mms instead of 160).
- exp granularity: one ACT op per m-chunk covering BOTH heads of the
  pair ([P,2,512] PSUM -> [P,2,512] bf16), so PSUM in scores stays at
  3x2 banks and ACT per-op fixed cost stays amortized.
- den handled by the v ones-column as before; den copy moved off ACT
  (vector), normalize mul on vector, partition broadcast on gpsimd.
"""

import sys

sys.path.insert(0, "/opt/trn_rl_repo")

from contextlib import ExitStack

import ml_dtypes
import numpy as np

import concourse.bass as bass
import concourse.tile as tile
from concourse import bacc, mybir
from concourse.bass_utils import run_bass_kernel_spmd

B, N, N2 = 4, 2048, 1024
DIM, H, HD = 1024, 16, 64
SCALE = HD ** -0.5
P = 128
R = 1024          # query rows per core
NCORES = 8
KO = DIM // P     # 8 contraction chunks
F32 = mybir.dt.float32
BF = mybir.dt.bfloat16
NPBF = ml_dtypes.bfloat16

M2C = 640         # compacted kv length (5 x 128); kept count must fit

TRACE = False


def build_kernel(m2c=M2C):
    mo_n = m2c // P
    nc = bacc.Bacc("TRN2", target_bir_lowering=False, debug=False,
                   num_devices=NCORES)
    xT = nc.dram_tensor("xT", [DIM, R], BF, kind="ExternalInput").ap()
    yT = nc.dram_tensor("yT", [DIM, m2c], BF, kind="ExternalInput").ap()
    wq = nc.dram_tensor("wq", [DIM, DIM], BF, kind="ExternalInput").ap()
    wk = nc.dram_tensor("wk", [DIM, DIM], BF, kind="ExternalInput").ap()
    wv = nc.dram_tensor("wv", [DIM, DIM], BF, kind="ExternalInput").ap()
    wp = nc.dram_tensor("wp", [DIM, DIM], BF, kind="ExternalInput").ap()
    keepc = nc.dram_tensor("keepc", [m2c], BF, kind="ExternalInput").ap()
    bp = nc.dram_tensor("bp", [DIM], F32, kind="ExternalInput").ap()
    out = nc.dram_tensor("out", [DIM, R], F32, kind="ExternalOutput").ap()

    # kv free-dim chunking for the k^T projection (<=512 per PSUM bank)
    kv_chunks = [(i, min(512, m2c - i)) for i in range(0, m2c, 512)]
    assert len(kv_chunks) <= 2

    with tile.TileContext(nc, pool_alloc_mode="queue") as tc, ExitStack() as ctx:
        persist = ctx.enter_context(tc.tile_pool(name="persist", bufs=1))
        qT = persist.tile([P, KO, R], BF)           # q^T, c-major
        kT = persist.tile([P, KO, m2c], BF)         # k^T, c-major
        vS = persist.tile([P, mo_n, H * 65], BF)    # v[m,c] + keep col / head
        attnT = persist.tile([P, KO, R], BF)        # attn out^T, c-major
        wp_r = persist.tile([P, KO, DIM], BF)       # Wproj, loaded during B
        kc = persist.tile([P, mo_n], BF)            # keep col, m-major
        bT = persist.tile([P, KO], F32)
        xT_r = [persist.tile([P, R], BF, name=f"xk{ko}") for ko in range(KO)]
        yT_r = [persist.tile([P, m2c], BF, name=f"yk{ko}") for ko in range(KO)]
        wv_t = persist.tile([P, KO, DIM], BF)       # full Wv staged

        wq3 = wq.rearrange("(ko p) c -> p ko c", p=P)
        wk3 = wk.rearrange("(ko p) c -> p ko c", p=P)
        wv3 = wv.rearrange("(ko p) c -> p ko c", p=P)
        wp3 = wp.rearrange("(ko p) c -> p ko c", p=P)
        xr3 = xT.rearrange("(ko p) f -> p ko f", p=P)
        yr3 = yT.rearrange("(ko p) f -> p ko f", p=P)

        wstg = ctx.enter_context(tc.tile_pool(name="wstg", bufs=2))
        pbe = ctx.enter_context(tc.tile_pool(name="pbe", bufs=2))
        outp = ctx.enter_context(tc.tile_pool(name="outp", bufs=3))
        # PSUM: ps2 3x[P,2,512] (6 banks) + psv 2x[P,512] (2 banks) = 8
        psS = ctx.enter_context(tc.tile_pool(name="psS", bufs=3, space="PSUM"))
        psV = ctx.enter_context(tc.tile_pool(name="psV", bufs=2, space="PSUM"))

        # ---- initial DMA issues, ordered by when compute needs them:
        # y+wk0 gate A2(0) (~5us), x+wq0 gate A1(0) (~12us), wv gates A3.
        # Only A2(0)'s inputs are issued before its emission so its DMA
        # watermark stays low (deps are per-queue counters).
        for ko in range(KO):
            nc.scalar.dma_start(yT_r[ko][:], yr3[:, ko])
        wk_t = wstg.tile([P, KO, 256], BF, tag="wk")
        nc.sync.dma_start(wk_t[:], wk3[:, :, 0:256])

        # PE warmup: ~5us of throwaway matmuls so the tensor clock is at
        # max p-state by the time the real work lands.
        warm = persist.tile([P, 512], BF)
        nc.vector.memset(warm[:], 0)
        psw = psV.tile([P, 512], F32, tag="psv", name="psw")
        for i in range(24):
            nc.tensor.matmul(psw[0:64, :], warm[:, 0:64], warm[:, :],
                             start=True, stop=True)

        def a2_kproj(co, wk_cur):
            """kT[:, co] <- Wk[:, co-block]^T @ y^T (8 ko accumulation)."""
            c2 = co % 2
            psk = psS.tile([P, 2, 512], F32, tag="ps2", name="psk")
            for ko in range(KO):
                for ci, (m0, mw) in enumerate(kv_chunks):
                    nc.tensor.matmul(
                        psk[:, ci, :mw], wk_cur[:, ko, c2 * P:(c2 + 1) * P],
                        yT_r[ko][:, m0:m0 + mw],
                        start=(ko == 0), stop=(ko == KO - 1))
                yield
            for ci, (m0, mw) in enumerate(kv_chunks):
                nc.vector.tensor_copy(kT[:, co, m0:m0 + mw], psk[:, ci, :mw])
            yield

        def a1_qproj(co, wq_cur):
            """qT[:, co] <- x @ Wq[:, co-block], both query halves."""
            psq = psS.tile([P, 2, 512], F32, tag="ps2", name="psq")
            for ko in range(KO):
                for nn2 in range(2):
                    nc.tensor.matmul(
                        psq[:, nn2], wq_cur[:, ko],
                        xT_r[ko][:, nn2 * 512:(nn2 + 1) * 512],
                        start=(ko == 0), stop=(ko == KO - 1))
                yield
            nc.vector.tensor_copy(qT[:, co, :], psq[:, :, :])
            yield

        def a3_vproj():
            """v = y @ Wv, m-major, y-stationary, c4-paired (N=512)."""
            for mo in range(mo_n):
                psv = psS.tile([P, 2, 512], F32, tag="ps2", name="psv3")
                for ko in range(KO):
                    for cp in range(2):
                        nc.tensor.matmul(
                            psv[:, cp], yT_r[ko][:, mo * P:(mo + 1) * P],
                            wv_t[:, ko, cp * 512:(cp + 1) * 512],
                            start=(ko == 0), stop=(ko == KO - 1))
                nc.vector.tensor_copy(
                    vH[:, mo, :, 0:64],
                    psv[:, :, :].rearrange("p c2 (h d) -> p (c2 h) d", d=64))

        ex_store = {}

        def scores_exp(co):
            """Row-tiled scores for head pair (2co, 2co+1) + exp; one
            yield per m-chunk so the zipper can pace the ACT stream."""
            exs = []
            for nn2 in range(2):
                ex = pbe.tile([P, mo_n, 2, 512], BF, tag="expS", bufs=4,
                              name="ex")
                exs.append(ex)
                for c in range(mo_n):
                    pss = psS.tile([P, 2, 512], F32, tag="ps2")
                    nc.tensor.matmul(
                        pss[:, 0], kT[0:64, co, c * P:(c + 1) * P],
                        qT[0:64, co, nn2 * 512:(nn2 + 1) * 512],
                        start=True, stop=True)
                    nc.tensor.matmul(
                        pss[:, 1], kT[64:128, co, c * P:(c + 1) * P],
                        qT[64:128, co, nn2 * 512:(nn2 + 1) * 512],
                        start=True, stop=True)
                    nc.scalar.activation(
                        ex[:, c], pss[:, :, :],
                        mybir.ActivationFunctionType.Exp, scale=float(SCALE))
                    yield
            ex_store[co] = tuple(exs)

        def attnv_norm(co, exs, nns=(0, 1)):
            """attn @ v + softmax normalize for the pair's instances."""
            for nn2 in nns:
                ex = exs[nn2]
                for h01 in range(2):
                    h = 2 * co + h01
                    ops = psV.tile([P, 512], F32, tag="psv")
                    for c in range(mo_n):
                        nc.tensor.matmul(
                            ops[0:65], vS[:, c, h * 65:(h + 1) * 65],
                            ex[:, c, h01],
                            start=(c == 0), stop=(c == mo_n - 1))
                    # single-copy PSUM evacuation so the bank frees after one
                    # DVE hop; recip/bcast/mul then run SBUF-side off the
                    # PSUM critical path
                    os = pbe.tile([65, 512], F32, tag="os")
                    nc.vector.tensor_copy(os[:], ops[0:65])
                    den = pbe.tile([1, 512], F32, tag="den")
                    nc.vector.tensor_copy(den[:], os[64:65])
                    rec = pbe.tile([1, 512], F32, tag="rec")
                    # approx recip must read SBUF, not PSUM (probed on HW),
                    # and is lane-locked so den must sit at partition 0
                    nc.vector.reciprocal_approx_fast(rec[:], den[:])
                    bc = pbe.tile([64, 512], F32, tag="bc")
                    nc.gpsimd.partition_broadcast(bc[:], rec[:])
                    nc.vector.tensor_mul(
                        attnT[h01 * 64:h01 * 64 + 64, co,
                              nn2 * 512:(nn2 + 1) * 512],
                        os[0:64], bc[:])
                    yield

        # ---- fused schedule. Per slot, the 10 score chunk-units are
        # zippered between the A2/A1/attnv matmul stream so the ACT exp
        # stream stays continuously fed (one exp ~1.1us per unit).
        wk_tiles = {0: wk_t}

        def a2_sched(co):
            """A2(co) generator, prefetching the wk chunk for co+2 first."""
            cq = co // 2
            if co % 2 == 0 and cq + 1 < 4:
                wk_nxt = wstg.tile([P, KO, 256], BF, tag="wk")
                nc.sync.dma_start(wk_nxt[:],
                                  wk3[:, :, (cq + 1) * 256:(cq + 2) * 256])
                wk_tiles[cq + 1] = wk_nxt
            return a2_kproj(co, wk_tiles[cq])

        def a1_sched(co):
            nonlocal wq_t
            if co + 1 < KO:
                wq_nxt = wstg.tile([P, KO, P], BF, tag="wq")
                nc.sync.dma_start(wq_nxt[:],
                                  wq3[:, :, (co + 1) * P:(co + 2) * P])
            g = a1_qproj(co, wq_t)
            if co + 1 < KO:
                wq_t = wq_nxt
            return g

        def drain(gen):
            for _ in gen:
                pass

        drain(a2_sched(0))

        # rest of the initial loads, issued after A2(0)'s emission so its
        # DMA-completion watermark only covers y+wk0
        nc.scalar.dma_start(kc[:], keepc.rearrange("(mo p) -> p mo", p=P))
        nc.scalar.dma_start(bT[:], bp.rearrange("(o p) -> p o", p=P))
        wq_t = wstg.tile([P, KO, P], BF, tag="wq")
        nc.sync.dma_start(wq_t[:], wq3[:, :, 0:P])
        for ko in range(KO):
            nc.gpsimd.dma_start(xT_r[ko][:], xr3[:, ko])
        vH = vS.rearrange("p mo (h s) -> p mo h s", s=65)
        for mo in range(mo_n):
            nc.gpsimd.tensor_copy(vH[:, mo, :, 64],
                                  kc[:, mo:mo + 1].to_broadcast([P, H]))

        drain(a2_sched(1))
        drain(a2_sched(2))
        drain(a1_sched(0))
        # wv issued after x so the x transfer isn't sharing the HBM line
        # with a weight block that isn't needed until A3 (~t30us)
        for i in range(4):
            nc.sync.dma_start(wv_t[:, :, i * 256:(i + 1) * 256],
                              wv3[:, :, i * 256:(i + 1) * 256])
        for co in range(KO):
            if co + 3 < KO:
                drain(a2_sched(co + 3))
            if co + 1 < KO:
                drain(a1_sched(co + 1))
            if co >= 4:                      # stream Wproj under B's shadow
                cw = co - 4
                nc.sync.dma_start(wp_r[:, :, cw * 256:(cw + 1) * 256],
                                  wp3[:, :, cw * 256:(cw + 1) * 256])
            drain(scores_exp(co))
            if co == 0:
                a3_vproj()
            if co > 0:
                drain(attnv_norm(co - 1, ex_store[co - 1]))

        def c_proj(nn2):
            """outT[c2, nn-half] = Wproj^T @ attnT + bias."""
            for c2o in range(KO):
                psc = psV.tile([P, 512], F32, tag="psv", name="psc")
                for co in range(KO):
                    nc.tensor.matmul(
                        psc[:], wp_r[:, co, c2o * P:(c2o + 1) * P],
                        attnT[:, co, nn2 * 512:(nn2 + 1) * 512],
                        start=(co == 0), stop=(co == KO - 1))
                osb = outp.tile([P, 512], F32, tag="osb")
                if c2o % 2 == 0:
                    # ACT is idle in phase C; alternating evac engines
                    # halves the psc bank-recycle latency chain
                    nc.scalar.activation(osb[:], psc[:],
                                         mybir.ActivationFunctionType.Identity,
                                         bias=bT[:, c2o:c2o + 1])
                else:
                    nc.vector.tensor_scalar_add(osb[:], psc[:],
                                                bT[:, c2o:c2o + 1])
                nc.sync.dma_start(
                    out[c2o * P:(c2o + 1) * P, nn2 * 512:(nn2 + 1) * 512],
                    osb[:])

        # tail: pair 7 nn0 first, then C(nn0) overlapping the nn1 norms
        drain(attnv_norm(7, ex_store[7], nns=(0,)))
        c_proj(0)
        drain(attnv_norm(7, ex_store[7], nns=(1,)))
        c_proj(1)

    nc.finalize()
    return nc


_NC = {}


def kernel(x, y, pad_mask, Wq, Wkv, Wproj, bproj):
    x = np.asarray(x, dtype=np.float32)
    y = np.asarray(y, dtype=np.float32)
    pad_mask = np.asarray(pad_mask)
    Wq = np.asarray(Wq, dtype=np.float32)
    Wkv = np.asarray(Wkv, dtype=np.float32)
    Wproj = np.asarray(Wproj, dtype=np.float32)
    bproj = np.asarray(bproj, dtype=np.float32)

    Wqb = np.ascontiguousarray(Wq.astype(NPBF))
    Wkb = np.ascontiguousarray(Wkv[:, :DIM].astype(NPBF))
    Wvb = np.ascontiguousarray(Wkv[:, DIM:].astype(NPBF))
    Wpb = np.ascontiguousarray(Wproj.astype(NPBF))

    # compact kv: gather kept rows per batch, pad with zeros to m2c
    keep_idx = [np.nonzero(pad_mask[b] != 0)[0] for b in range(B)]
    max_kept = max(len(i) for i in keep_idx)
    m2c = M2C if max_kept <= M2C else N2
    yc = np.zeros((B, m2c, DIM), dtype=np.float32)
    keepc = np.zeros((B, m2c), dtype=NPBF)
    for b in range(B):
        k = len(keep_idx[b])
        yc[b, :k] = y[b][keep_idx[b]]
        keepc[b, :k] = 1.0

    xTb = [np.ascontiguousarray(x[b, half * R:(half + 1) * R, :].T.astype(NPBF))
           for b in range(B) for half in range(2)]
    yTb = [np.ascontiguousarray(yc[b].T.astype(NPBF)) for b in range(B)]

    in_maps = []
    for c in range(NCORES):
        b, half = c // 2, c % 2
        in_maps.append({
            "xT": xTb[c],
            "yT": yTb[b],
            "wq": Wqb, "wk": Wkb, "wv": Wvb, "wp": Wpb,
            "keepc": keepc[b],
            "bp": bproj,
        })

    if m2c not in _NC:
        _NC[m2c] = build_kernel(m2c)

    res = run_bass_kernel_spmd(_NC[m2c], in_maps, core_ids=list(range(NCORES)),
                               trace=TRACE)
    if TRACE:
        kernel.last_results = res

    full = np.empty((B, N, DIM), dtype=np.float32)
    for c in range(NCORES):
        b, half = c // 2, c % 2
        full[b, half * R:(half + 1) * R, :] = res.results[c]["out"].T
    return full
